# revision 1
# baseline (speedup 1.0000x reference)
"""BiLSTM language-model kernel for 8 Trainium2 NeuronCores.

Reference computation (backward LSTM direction is dead code in the reference):
    x  = emb[input]                          # (B=8, T=512, E=512)
    xg = x @ W_ih_fwd.T + b_ih + b_hh        # (T, B, 4H)
    h  = LSTM-scan(xg, W_hh_fwd)             # (T, B, H)
    out = h @ W_out.T + b_out                # (B, T, V=32000)

Distribution strategy:
  - Embedding lookup: host-side (pure indexed copy of inputs).
  - xg GEMM: sharded over T across the 8 cores (bf16), ONE AllGather in a
    [p, m, t, b] layout so both the phase-1 writes and the scan's chunked
    SBUF loads are contiguous (no per-step DMA at all).
  - LSTM scan: replicated on all 8 cores. Per step: 64 LDW+MM pairs
    (bf16 W_hh stationary, FWL) in gate-group order (g, i, f, o) so the
    tanh(g)/sigmoid(i) chain hides under the MM burst and only the
    o-gate + tanh(c) path trails the burst. h is written once per step
    as bf16 directly into the out-GEMM staging tile.
  - Output GEMM: vocab-sharded (4000 rows/core), bf16, N=1000 moving
    operand, one MM per scan step interleaved into the tail bubble.
"""

import os
import numpy as np
import ml_dtypes

import concourse.bass as bass
import concourse.tile as tile
from concourse import bacc, mybir
from concourse.bass_utils import run_bass_kernel_spmd

F32 = mybir.dt.float32
BF16 = mybir.dt.bfloat16
AF = mybir.ActivationFunctionType
ALU = mybir.AluOpType

N_CORES = 8
B, T, E, H, V = 8, 512, 512, 512, 32000
G = 4 * H                   # 2048 gate rows
NM = G // 128               # 16 gate M-tiles
NK = H // 128               # 4 contraction K-tiles
TC = T // N_CORES           # 64 timesteps per core for the xg GEMM
VC = V // N_CORES           # 4000 vocab rows per core
VCH = 8                     # vocab chunks in output GEMM
VN = VC // VCH              # 1000 vocab per chunk
NBT = (B * T) // 128        # 32 bt-tiles in the output GEMM

_T_BUILD = int(os.environ.get("BILSTM_T_BUILD", "512"))  # dev knob: scan length

# gate m-tile group order: g(0:4) i(4:8) f(8:12) o(12:16) — g first so its
# tanh can start while later groups' matmuls still stream; o last.
_PERM = np.concatenate([np.arange(2 * H, 3 * H), np.arange(0, H),
                        np.arange(H, 2 * H), np.arange(3 * H, 4 * H)])
GG, GI, GF, GO = 0, 1, 2, 3  # group index = m-tiles 4g..4g+4

_CACHE = {}


def _wire_ntff_hook():
    """The agent image's antenv lacks axon_hooks; synthesize it so
    run_bass_kernel_spmd(trace=True) can capture NTFF profiles."""
    import sys
    import types
    try:
        from antenv.axon_hooks import get_axon_ntff_profile_hook  # noqa: F401
        return
    except ImportError:
        pass
    try:
        import antenv
        from trn_agent_boot.trn_boot import _ntff_profile_via_ctypes
        mod = types.ModuleType("antenv.axon_hooks")
        _store = [None]
        mod.set_axon_ntff_profile_hook = lambda h: _store.__setitem__(0, h)
        mod.get_axon_ntff_profile_hook = lambda: _store[0]
        sys.modules["antenv.axon_hooks"] = mod
        antenv.axon_hooks = mod
        mod.set_axon_ntff_profile_hook(
            _ntff_profile_via_ctypes("/opt/axon/libaxon_pjrt.so"))
    except Exception:
        pass


_wire_ntff_hook()


def _build():
    if "nc" in _CACHE:
        return _CACHE["nc"]
    nc = bacc.Bacc("TRN2", target_bir_lowering=False, debug=False,
                   num_devices=N_CORES)

    # ---- DRAM I/O ----
    xt_dram = nc.dram_tensor("xt", [E, TC * B], BF16, kind="ExternalInput")
    wih_dram = nc.dram_tensor("wih", [E, G], BF16, kind="ExternalInput")
    whh_dram = nc.dram_tensor("whh", [H, G], BF16, kind="ExternalInput")
    bg_dram = nc.dram_tensor("bg", [128, NM], F32, kind="ExternalInput")
    wout_dram = nc.dram_tensor("wout", [H, VC], BF16, kind="ExternalInput")
    bout_dram = nc.dram_tensor("bout", [128, VC], F32, kind="ExternalInput")
    out_dram = nc.dram_tensor("out", [B, T, VC], BF16, kind="ExternalOutput")

    # xg intermediate, layout [p, m, t_local, b]: contiguous per-partition
    # runs for both the phase-1 writes (1KB) and the scan chunk reads (16KB)
    xg_mine = nc.dram_tensor("xg_mine", [128, NM, TC, B], BF16)
    xg_all = nc.dram_tensor("xg_all", [N_CORES, 128, NM, TC, B], BF16,
                            addr_space="Shared")

    n_chunks = (_T_BUILD + TC - 1) // TC

    with tile.TileContext(nc) as tc:
        with (
            tc.tile_pool(name="wbig", bufs=1) as wbig,      # weights
            tc.tile_pool(name="wsmall", bufs=1) as wsmall,
            tc.tile_pool(name="state", bufs=1) as statep,   # scan state
            tc.tile_pool(name="hs", bufs=NBT) as hsp,       # h staging (bf16)
            tc.tile_pool(name="xgc", bufs=3) as xgcp,       # xg chunk buffers
            tc.tile_pool(name="xgst", bufs=3) as xgst,      # phase1 staging
            tc.tile_pool(name="gt", bufs=4) as gtp,         # gate tiles
            tc.tile_pool(name="ovec", bufs=3) as ovec,      # out staging
            tc.tile_pool(name="psg", bufs=5, space="PSUM") as psgp,  # gemm+phase1
            tc.tile_pool(name="psa", bufs=1, space="PSUM") as ps_a,  # g
            tc.tile_pool(name="psb", bufs=1, space="PSUM") as ps_b,  # i+f
            tc.tile_pool(name="psd", bufs=1, space="PSUM") as ps_d,  # o
        ):

            # ================= phase 0: weight loads =================
            # sync queue: phase-1 inputs + later the xg chunk loads.
            # scalar queue: everything the scan + out-GEMM needs (whh, bg,
            # wout, bout), in consumption order; the per-ot output DMAs
            # join this queue later.
            wih = [wbig.tile([128, G], BF16, tag=f"wih{k}", name=f"wih{k}")
                   for k in range(NK)]
            xt = [wsmall.tile([128, TC * B], BF16, tag=f"xt{k}", name=f"xt{k}")
                  for k in range(NK)]
            for k in range(NK):
                nc.sync.dma_start(xt[k][:], xt_dram[128 * k:128 * (k + 1), :])
                nc.sync.dma_start(wih[k][:], wih_dram[128 * k:128 * (k + 1), :])
            whh = wsmall.tile([128, NK, G], BF16)
            nc.scalar.dma_start(whh[:], whh_dram[:].rearrange("(k p) g -> p k g", p=128))
            bg = wsmall.tile([128, NM], F32)
            nc.scalar.dma_start(bg[:], bg_dram[:])
            wout = []
            for v in range(VCH):
                wt = wbig.tile([128, NK, VN], BF16, tag=f"wout{v}",
                               name=f"wout{v}")
                nc.scalar.dma_start(
                    wt[:],
                    wout_dram[:, VN * v:VN * (v + 1)].rearrange(
                        "(k p) v -> p k v", p=128))
                wout.append(wt)
            bout = wsmall.tile([128, VC], F32)
            nc.scalar.dma_start(bout[:], bout_dram[:])

            # ================= phase 1: xg GEMM (my T-chunk) =================
            for m in range(NM):
                ps = psgp.tile([128, TC * B], F32, tag="psg", name=f"xg_ps{m}")
                for k in range(NK):
                    nc.tensor.matmul(
                        ps[:], wih[k][:, 128 * m:128 * (m + 1)], xt[k][:],
                        start=(k == 0), stop=(k == NK - 1))
                st = xgst.tile([128, TC * B], BF16, tag="xgst",
                                name=f"xg_st{m}")
                nc.scalar.activation(st[:], ps[:], AF.Identity,
                                     bias=bg[:, m:m + 1])
                nc.sync.dma_start(
                    xg_mine[:, m, :, :],
                    st[:].rearrange("p (t b) -> p t b", b=B))

            # ================= phase 2: AllGather xg (one shot) ==============
            nc.gpsimd.collective_compute(
                "AllGather", ALU.bypass,
                ins=[xg_mine[:]], outs=[xg_all[:]],
                replica_groups=[list(range(N_CORES))])

            # ================= phase 3+4: LSTM scan + interleaved out-GEMM ===
            c_t = statep.tile([128, NK, B], F32)
            t1 = statep.tile([128, NK, B], F32)
            t2 = statep.tile([128, NK, B], F32)
            tnc = statep.tile([128, NK, B], F32)
            h0 = statep.tile([128, NK, B], BF16)
            nc.vector.memset(c_t[:], 0.0)
            nc.vector.memset(h0[:].bitcast(mybir.dt.uint16), 0)

            hs = [hsp.tile([128, NK, 128], BF16, tag="hs", name=f"hs{j}")
                  for j in range(NBT)]
            for hst in hs:
                nc.vector.memset(hst[:].bitcast(mybir.dt.uint16), 0)

            # xg chunk prefetch (sync queue drains as bufs free)
            xgc = []
            for ccn in range(n_chunks):
                xt_c = xgcp.tile([128, NM, TC, B], BF16, tag="xgc",
                                 name=f"xgc{ccn}")
                for s8 in range(0, TC, 8):
                    nc.sync.dma_start(xt_c[:, :, s8:s8 + 8, :],
                                      xg_all[ccn][:, :, s8:s8 + 8, :])
                xgc.append(xt_c)

            gemm_ps = {}   # v-chunk psum tiles for the interleaved out-GEMM

            def emit_gemm_mm(j, v, k):
                if k == 0:
                    gemm_ps[(j, v)] = psgp.tile(
                        [128, VN], F32, tag="psg", name=f"gps{j}_{v}")
                nc.tensor.matmul(
                    gemm_ps[(j, v)][:], hs[j][:, k, :],
                    wout[v][:, k, :],
                    start=(k == 0), stop=(k == NK - 1),
                    skip_group_check=True)

            ot_blk = {}

            def emit_gemm_out(j, v):
                ps = gemm_ps.pop((j, v))
                if v == 0:
                    ot_blk[j] = ovec.tile([128, VC], BF16, tag="ot",
                                          name=f"ot{j}")
                ot = ot_blk[j]
                nc.vector.tensor_add(ot[:, VN * v:VN * (v + 1)], ps[:],
                                     bout[:, VN * v:VN * (v + 1)])
                if v == VCH - 1:
                    dst = out_dram[:, 16 * j:16 * (j + 1), :]
                    nc.scalar.dma_start(dst.rearrange("b t v -> t b v"),
                                        ot_blk.pop(j)[:])

            FUNCS = [AF.Tanh, AF.Sigmoid, AF.Sigmoid, AF.Sigmoid]
            for t in range(_T_BUILD):
                cc, tl = t // TC, t % TC
                if t == 0:
                    def h_ap(k):
                        return h0[:, k, :]
                else:
                    jp, op = (t - 1) // 16, (t - 1) % 16
                    def h_ap(k, _j=jp, _o=op):
                        return hs[_j][:, k, B * _o:B * (_o + 1)]

                # ---- 64 LDW+MM pairs, groups (g, i+f merged, o) ----
                ps_g = ps_a.tile([128, 4, B], F32, tag="psG", name=f"psG_{t}")
                ps_if = ps_b.tile([128, 8, B], F32, tag="psIF", name=f"psIF_{t}")
                ps_o = ps_d.tile([128, 4, B], F32, tag="psO", name=f"psO_{t}")

                def mm_dst(m):
                    if m < 4:
                        return ps_g[:, m, :]
                    if m < 12:
                        return ps_if[:, m - 4, :]
                    return ps_o[:, m - 12, :]
                for m in range(NM):
                    for k in range(NK):
                        nc.tensor.matmul(
                            mm_dst(m),
                            whh[:, k, 128 * m:128 * (m + 1)],
                            h_ap(k),
                            start=(k == 0), stop=(k == NK - 1))

                # ---- elementwise chain ----
                # DVE: add_g, add_if, t1, add_o, t2, c, h
                # ACT: tanh_g, sig_if, sig_o, tanh_c
                gtg = gtp.tile([128, 4, B], F32, tag="gG", name=f"gG_{t}")
                nc.vector.tensor_add(gtg[:], ps_g[:], xgc[cc][:, 0:4, tl, :])
                nc.scalar.activation(gtg[:], gtg[:], AF.Tanh)
                gtif = gtp.tile([128, 8, B], F32, tag="gIF", name=f"gIF_{t}")
                nc.vector.tensor_add(gtif[:], ps_if[:], xgc[cc][:, 4:12, tl, :])
                nc.scalar.activation(gtif[:], gtif[:], AF.Sigmoid)
                nc.vector.tensor_mul(t1[:], gtif[:, 0:4, :], gtg[:])
                gto = gtp.tile([128, 4, B], F32, tag="gO", name=f"gO_{t}")
                nc.vector.tensor_add(gto[:], ps_o[:], xgc[cc][:, 12:16, tl, :])
                nc.vector.tensor_mul(t2[:], gtif[:, 4:8, :], c_t[:])
                nc.vector.tensor_add(c_t[:], t1[:], t2[:])
                nc.scalar.activation(gto[:], gto[:], AF.Sigmoid)
                nc.scalar.activation(tnc[:], c_t[:], AF.Tanh)
                j, o = t // 16, t % 16
                nc.vector.tensor_mul(hs[j][:, :, B * o:B * (o + 1)],
                                     gto[:], tnc[:])

                # interleave the previous bt-tile's output GEMM (1 MM/step)
                jj = t // 16 - 1
                if 0 <= jj < NBT:
                    idx = t % 16
                    for pair in (2 * idx, 2 * idx + 1):
                        v, k = divmod(pair, NK)
                        emit_gemm_mm(jj, v, k)
                        if k == NK - 1:
                            emit_gemm_out(jj, v)

            # tail: last bt-tile (and any skipped when _T_BUILD < T)
            done_j = max(0, _T_BUILD // 16 - 1)
            for j in range(done_j, NBT):
                for v in range(VCH):
                    for k in range(NK):
                        emit_gemm_mm(j, v, k)
                    emit_gemm_out(j, v)

    nc.compile()
    _CACHE["nc"] = nc
    return nc


def kernel(**inputs) -> np.ndarray:
    inp = np.asarray(inputs["input"])
    emb = np.asarray(inputs["emb"], dtype=np.float32)
    W_ih = np.asarray(inputs["W_ih_fwd"], dtype=np.float32)
    b_ih = np.asarray(inputs["b_ih_fwd"], dtype=np.float32)
    W_hh = np.asarray(inputs["W_hh_fwd"], dtype=np.float32)
    b_hh = np.asarray(inputs["b_hh_fwd"], dtype=np.float32)
    W_out = np.asarray(inputs["W_out"], dtype=np.float32)
    b_out = np.asarray(inputs["b_out"], dtype=np.float32)

    nc = _build()

    # host-side input prep
    x = emb[inp]                                   # (B, T, E)
    bf = ml_dtypes.bfloat16
    wihT = np.ascontiguousarray(W_ih[_PERM].T).astype(bf)   # (E, G)
    whhT = np.ascontiguousarray(W_hh[_PERM].T).astype(bf)
    bgv = (b_ih + b_hh)[_PERM].reshape(NM, 128).T.copy()    # (128, NM)

    in_maps = []
    for c in range(N_CORES):
        xc = x[:, TC * c:TC * (c + 1), :]          # (B, TC, E)
        xtc = np.ascontiguousarray(
            xc.transpose(2, 1, 0).reshape(E, TC * B)).astype(bf)
        wo = np.ascontiguousarray(W_out[VC * c:VC * (c + 1)].T).astype(bf)
        bo = np.tile(b_out[VC * c:VC * (c + 1)][None, :], (128, 1))
        in_maps.append({
            "xt": xtc, "wih": wihT, "whh": whhT, "bg": bgv,
            "wout": wo, "bout": np.ascontiguousarray(bo),
        })

    res = run_bass_kernel_spmd(
        nc, in_maps, core_ids=list(range(N_CORES)),
        trace=bool(int(os.environ.get("BILSTM_TRACE", "0"))))
    _CACHE["last_res"] = res
    out = np.concatenate([res.results[c]["out"] for c in range(N_CORES)], axis=2)
    return out.astype(np.float32)

